# revision 1
# baseline (speedup 1.0000x reference)
"""2-layer GraphSAGE (mean aggregation) on 8 trn2 NeuronCores via Bass/Tile.

Strategy (matches the sharding hint):
  - Nodes are row-sharded across the 8 cores (6250 rows each); edges are
    partitioned by destination core.
  - Per core, edges are grouped by 128-node destination block.  Messages
    x[src] are fetched with InstDMAGatherAnt (edge-major tiles of 128) from a
    bf16 256B-padded copy of the features, and the segment-sum is computed as
    a one-hot matmul on the tensor engine (bf16, 1 cycle/row):
        aggT[64f, 128d] += msgs[128e, 64f].T @ onehot[128e, 128d]
    where onehot[e, d] = (d == dst_local[e]) is built on the vector engine
    from a broadcast iota with one tensor_scalar(is_equal) op per tile.
    The exact f32 1/deg scaling is applied at PSUM->SBUF copy time via a
    host-built [64, n] broadcast table (elementwise mult on DVE).
  - The 64x64 weights are replicated; the dense phase runs feature-major in
    f32 on rotating [64, 512] group buffers.
  - h = tanh(layer1) chunks (bf16-padded) are AllGathered between layers.
  - dma_gather indices are int16, so each gather call reads one of two row
    regions of the source, split at a core boundary so lo/hi membership is
    identical for the x-space and the padded h-space.
"""

import numpy as np
import ml_dtypes

import concourse.bacc as bacc
import concourse.mybir as mybir
import concourse.tile as tile
from concourse.bass_utils import run_bass_kernel_spmd

P = 128
D = 64
F32 = mybir.dt.float32
BF16 = mybir.dt.bfloat16
I16 = mybir.dt.int16
BF = ml_dtypes.bfloat16


class Cfg:
    def __init__(self, N, n_cores=8, chunk=64, msgs_bufs=4):
        assert N % n_cores == 0
        self.N = N
        self.n_cores = n_cores
        self.n_own = N // n_cores
        self.nblk = -(-self.n_own // P)
        self.n_own_pad = self.nblk * P
        self.n_pad_all = self.n_own_pad * n_cores
        # lo/hi split at a core boundary so that edge region membership is
        # identical for x-space (N rows) and padded h-space (n_pad_all rows).
        c = n_cores // 2
        while self.N - c * self.n_own > 32768 or self.n_pad_all - c * self.n_own_pad > 32768:
            c += 1
        assert c * self.n_own <= 32768 and c * self.n_own_pad <= 32768
        self.split_core = c
        self.split = c * self.n_own
        self.split_pad = c * self.n_own_pad
        self.chunk = chunk
        self.msgs_bufs = msgs_bufs


class Meta:
    pass


def _wrap16(v):
    """slot i -> [i % 16, i // 16] layout used by dma_gather idx tables."""
    assert v.shape[0] % 16 == 0
    return np.ascontiguousarray(v.reshape(-1, 16).T)


def preprocess(edge_index, cfg):
    """Partition/group edges; build per-core gather index + onehot tables."""
    src = np.asarray(edge_index[0], dtype=np.int64)
    dst = np.asarray(edge_index[1], dtype=np.int64)
    E = src.shape[0]
    NC, NBLK = cfg.n_cores, cfg.nblk

    cnt = np.bincount(dst, minlength=cfg.N).astype(np.float32)
    inv = (1.0 / np.maximum(cnt, 1.0)).astype(np.float32)

    core = dst // cfg.n_own
    dstl = dst - core * cfg.n_own
    blk = dstl // P
    inb = dstl - blk * P
    region = (src >= cfg.split).astype(np.int64)

    key = ((core * NBLK) + blk) * 2 + region
    ngroups = NC * NBLK * 2
    gcnt = np.bincount(key, minlength=ngroups).reshape(NC, NBLK, 2)
    # uniform (max over cores) tile counts per (block, region)
    TL = np.maximum(1, -(-gcnt[:, :, 0].max(axis=0) // P))
    TH = np.maximum(1, -(-gcnt[:, :, 1].max(axis=0) // P))
    lo_off = np.concatenate([[0], np.cumsum(TL)])
    hi_off = np.concatenate([[0], np.cumsum(TH)])
    TLT, THT = int(lo_off[-1]), int(hi_off[-1])
    T_ALL = TLT + THT

    # rank of each edge within its (core, blk, region) group
    order = np.argsort(key, kind="stable")
    gstart = np.concatenate([[0], np.cumsum(np.bincount(key, minlength=ngroups))])[:-1]
    rank = np.empty(E, dtype=np.int64)
    rank[order] = np.arange(E) - gstart[key[order]]

    # slot within region (tiles of 128)
    reg_base = np.where(region == 0, lo_off[blk], hi_off[blk])
    slot = reg_base * P + rank

    # h-space (padded) position of each source node
    pos = (src // cfg.n_own) * cfg.n_own_pad + (src % cfg.n_own)

    meta = Meta()
    meta.cfg = cfg
    meta.TL, meta.TH = TL, TH
    meta.TLT, meta.THT, meta.T_ALL = TLT, THT, T_ALL
    meta.block_tiles = [
        list(range(int(lo_off[b]), int(lo_off[b + 1])))
        + [TLT + t for t in range(int(hi_off[b]), int(hi_off[b + 1]))]
        for b in range(NBLK)
    ]

    # per-core tables
    meta.idx = []   # [128, 8*(TLT+THT)*2] int16 : l1lo | l1hi | l2lo | l2hi
    meta.dstf = []  # [128, T_ALL] f32
    meta.invb = []  # [64, n_own_pad] f32 : 1/deg broadcast down 64 partitions
    for k in range(NC):
        m = core == k
        sl = slot[m]
        rg = region[m]
        s_lo, s_hi = sl[rg == 0], sl[rg == 1]
        i1lo = np.zeros(TLT * P, np.int16)
        i1hi = np.zeros(THT * P, np.int16)
        i2lo = np.zeros(TLT * P, np.int16)
        i2hi = np.zeros(THT * P, np.int16)
        i1lo[s_lo] = src[m][rg == 0]
        i1hi[s_hi] = src[m][rg == 1] - cfg.split
        i2lo[s_lo] = pos[m][rg == 0]
        i2hi[s_hi] = pos[m][rg == 1] - cfg.split_pad
        w = np.concatenate(
            [_wrap16(a) for a in (i1lo, i1hi, i2lo, i2hi)], axis=1)
        # the gather ucode reads each Q7 core's idx stripe from its own
        # 16-partition group -> replicate 8x down the partition axis
        meta.idx.append(np.ascontiguousarray(np.tile(w, (8, 1))))

        df = np.full(T_ALL * P, -1.0, np.float32)
        gs = np.where(rg == 0, 0, TLT * P) + sl
        df[gs] = inb[m].astype(np.float32)
        meta.dstf.append(np.ascontiguousarray(df.reshape(T_ALL, P).T))

        iv = np.ones(cfg.n_own_pad, np.float32)
        iv[:cfg.n_own] = inv[k * cfg.n_own:(k + 1) * cfg.n_own]
        meta.invb.append(np.ascontiguousarray(np.tile(iv, (D, 1))))

    meta.idx_off = [0, TLT * 8, (TLT + THT) * 8, (2 * TLT + THT) * 8]

    # gather calls: (region, t0, ntiles, first_block), interleaved by the
    # first destination block each chunk serves.
    def chunks(T_total, offs):
        out = []
        t0 = 0
        while t0 < T_total:
            nt = min(cfg.chunk, T_total - t0)
            fb = int(np.searchsorted(offs, t0, side="right") - 1)
            out.append((t0, nt, fb))
            t0 += nt
        return out

    calls = [(0, t0, nt, fb) for (t0, nt, fb) in chunks(TLT, lo_off)]
    calls += [(1, t0, nt, fb) for (t0, nt, fb) in chunks(THT, hi_off)]
    calls.sort(key=lambda c: (c[3], c[0]))
    meta.calls = calls
    return meta


GCOL = 512  # dense-phase group width (one PSUM bank)


def build_program(meta, one_core=False,
                  parts=("gather", "agg", "dense", "store", "collective"),
                  reps=1):
    cfg = meta.cfg
    NC, NBLK = cfg.n_cores, cfg.nblk
    NP = cfg.n_own_pad
    BPG = GCOL // P  # blocks per dense group
    nc = bacc.Bacc(
        "TRN2", target_bir_lowering=False, debug=False,
        num_devices=1 if one_core else NC,
    )

    xp_dr = nc.dram_tensor("xp", [cfg.N, P], BF16, kind="ExternalInput")
    xoT_dr = nc.dram_tensor("xoT", [D, NP], F32, kind="ExternalInput")
    idx_dr = nc.dram_tensor("idx", list(meta.idx[0].shape), I16, kind="ExternalInput")
    dstf_dr = nc.dram_tensor("dstf", [P, meta.T_ALL], F32, kind="ExternalInput")
    invb_dr = nc.dram_tensor("invb", [D, NP], F32, kind="ExternalInput")
    wl1_dr = nc.dram_tensor("wl1t", [D, D], F32, kind="ExternalInput")
    wr1_dr = nc.dram_tensor("wr1t", [D, D], F32, kind="ExternalInput")
    wl2_dr = nc.dram_tensor("wl2t", [D, D], F32, kind="ExternalInput")
    wr2_dr = nc.dram_tensor("wr2t", [D, D], F32, kind="ExternalInput")
    b1_dr = nc.dram_tensor("b1", [D, 1], F32, kind="ExternalInput")
    b2_dr = nc.dram_tensor("b2", [D, 1], F32, kind="ExternalInput")
    iota_dr = nc.dram_tensor("iota", [P, P], BF16, kind="ExternalInput")
    id_dr = nc.dram_tensor("ident", [D, D], F32, kind="ExternalInput")
    out_dr = nc.dram_tensor("out", [NP, D], F32, kind="ExternalOutput")

    with tile.TileContext(nc) as tc:
        with (
            tc.tile_pool(name="const", bufs=1) as cpool,
            tc.tile_pool(name="big", bufs=1) as bpool,
            tc.tile_pool(name="msgs", bufs=cfg.msgs_bufs) as mpool,
            tc.tile_pool(name="idxp", bufs=4) as ipool,
            tc.tile_pool(name="ohp", bufs=12) as ohpool,
            tc.tile_pool(name="grp", bufs=2) as gpool,
            tc.tile_pool(name="psA", bufs=4, space="PSUM") as psA,
            tc.tile_pool(name="psZ", bufs=2, space="PSUM") as psZ,
            tc.tile_pool(name="psT", bufs=2, space="PSUM") as psT,
            tc.tile_pool(name="dram", bufs=1, space="DRAM") as dpool,
        ):
            def load(pool, dr, shape, name, dt=F32, tag=""):
                t = pool.tile(shape, dt, name=name, tag=tag or name)
                nc.sync.dma_start(out=t, in_=dr.ap())
                return t

            iota_sb = load(cpool, iota_dr, [P, P], "iota_sb", dt=BF16)
            ident_sb = load(cpool, id_dr, [D, D], "ident_sb")
            wl1_sb = load(cpool, wl1_dr, [D, D], "wl1_sb")
            wr1_sb = load(cpool, wr1_dr, [D, D], "wr1_sb")
            wl2_sb = load(cpool, wl2_dr, [D, D], "wl2_sb")
            wr2_sb = load(cpool, wr2_dr, [D, D], "wr2_sb")
            b1_sb = load(cpool, b1_dr, [D, 1], "b1_sb")
            b2_sb = load(cpool, b2_dr, [D, 1], "b2_sb")
            dstf_sb = load(bpool, dstf_dr, [P, meta.T_ALL], "dstf_sb")
            invb_sb = load(bpool, invb_dr, [D, NP], "invb_sb")
            xoT_sb = load(bpool, xoT_dr, [D, NP], "xoT_sb")
            hT_sb = bpool.tile([D, NP], F32, name="hT_sb")
            nodeh_sb = bpool.tile([P, NBLK * P], BF16, name="nodeh_sb")
            nodeo_sb = bpool.tile([P, NBLK * D], F32, name="nodeo_sb")
            # zero the bf16 pad columns once (cols [b*128+64, b*128+128))
            nc.vector.memset(nodeh_sb, 0.0)

            for rep in range(reps):
              h_chunk = dpool.tile([NP, P], BF16, name=f"h_chunk_{rep}", tag=f"hc{rep}")
              h_full = dpool.tile([cfg.n_pad_all, P], BF16, name=f"h_full_{rep}",
                                  tag=f"hf{rep}", addr_space="Shared")
              for layer in range(2):
                if layer == 0:
                    src_lo = xp_dr.ap()[0:cfg.split, :]
                    src_hi = xp_dr.ap()[cfg.split:cfg.N, :]
                    off_lo, off_hi = meta.idx_off[0], meta.idx_off[1]
                else:
                    src_lo = h_full[0:cfg.split_pad, :]
                    src_hi = h_full[cfg.split_pad:cfg.n_pad_all, :]
                    off_lo, off_hi = meta.idx_off[2], meta.idx_off[3]

                # ---- gather messages (bf16, 256B rows) ----
                tsrc = {}
                for ci, (rg, t0, ntile, _fb) in enumerate(meta.calls):
                    mt = mpool.tile([P, cfg.chunk, P], BF16, tag="msgs",
                                    name=f"m_{layer}_{ci}")
                    if "gather" in parts:
                        it = ipool.tile([P, cfg.chunk * 8], I16, tag="idx",
                                        name=f"i_{layer}_{ci}")
                        cols = ntile * 8
                        coff = (off_lo if rg == 0 else off_hi) + t0 * 8
                        nc.sync.dma_start(out=it[:, :cols],
                                          in_=idx_dr.ap()[:, coff:coff + cols])
                        nc.gpsimd.dma_gather(
                            mt[:, :ntile, :],
                            src_lo if rg == 0 else src_hi,
                            it[:, :cols],
                            num_idxs=ntile * P,
                            num_idxs_reg=ntile * P,
                            elem_size=P,
                            single_packet=False,
                        )
                    base = t0 if rg == 0 else meta.TLT + t0
                    for j in range(ntile):
                        tsrc[base + j] = (mt, j)

                # ---- blocks: onehot matmul segment-sum + dense per group ----
                if layer == 0:
                    wl_sb, wr_sb, bb_sb = wl1_sb, wr1_sb, b1_sb
                    own_sb = xoT_sb
                    func = mybir.ActivationFunctionType.Tanh
                else:
                    wl_sb, wr_sb, bb_sb = wl2_sb, wr2_sb, b2_sb
                    own_sb = hT_sb
                    func = mybir.ActivationFunctionType.Identity

                ngrp = -(-NBLK // BPG)
                for g in range(ngrp if "agg" in parts else 0):
                    b0 = g * BPG
                    nb = min(BPG, NBLK - b0)
                    w = nb * P
                    aggT = gpool.tile([D, GCOL], F32, tag="aggT",
                                      name=f"agg_{rep}_{layer}_{g}")
                    for bi in range(nb):
                        b = b0 + bi
                        ps = psA.tile([D, P], F32, tag="agg", name=f"ps_{layer}_{b}")
                        gts = meta.block_tiles[b]
                        for j, gt in enumerate(gts):
                            oh = ohpool.tile([P, P], BF16, tag="oh",
                                             name=f"oh_{layer}_{b}_{j}")
                            nc.vector.tensor_scalar(
                                out=oh, in0=iota_sb,
                                scalar1=dstf_sb[:, gt:gt + 1],
                                scalar2=None,
                                op0=mybir.AluOpType.is_equal,
                            )
                            mt, lt = tsrc[gt]
                            nc.tensor.matmul(
                                ps, lhsT=mt[:, lt, 0:D], rhs=oh,
                                start=(j == 0), stop=(j == len(gts) - 1),
                            )
                        # exact mean scaling: psum * (1/deg) broadcast table
                        nc.vector.tensor_tensor(
                            out=aggT[:, bi * P:(bi + 1) * P], in0=ps,
                            in1=invb_sb[:, b * P:(b + 1) * P],
                            op=mybir.AluOpType.mult,
                        )
                    if "dense" not in parts:
                        continue
                    zp = psZ.tile([D, GCOL], F32, tag="z", name=f"z_{layer}_{g}")
                    nc.tensor.matmul(zp[:, :w], lhsT=wl_sb, rhs=aggT[:, :w],
                                     start=True, stop=False)
                    nc.tensor.matmul(zp[:, :w], lhsT=wr_sb,
                                     rhs=own_sb[:, b0 * P:b0 * P + w],
                                     start=False, stop=True)
                    if layer == 0:
                        outT = hT_sb
                        nc.scalar.activation(out=hT_sb[:, b0 * P:b0 * P + w],
                                             in_=zp[:, :w], func=func,
                                             bias=bb_sb[:, 0:1], scale=1.0)
                    else:
                        outT = gpool.tile([D, GCOL], F32, tag="outT",
                                          name=f"oT_{rep}_{g}")
                        nc.scalar.activation(out=outT[:, :w], in_=zp[:, :w],
                                             func=func, bias=bb_sb[:, 0:1],
                                             scale=1.0)
                    if "store" not in parts:
                        continue
                    for bi in range(nb):
                        b = b0 + bi
                        tp = psT.tile([P, D], F32, tag="tr", name=f"tp_{layer}_{b}")
                        sl = (slice(b * P, b * P + P) if layer == 0
                              else slice(bi * P, bi * P + P))
                        nc.tensor.transpose(out=tp, in_=outT[:, sl],
                                            identity=ident_sb)
                        if layer == 0:
                            # bf16 padded node-major h rows
                            nc.scalar.copy(out=nodeh_sb[:, b * P:b * P + D],
                                           in_=tp)
                        else:
                            nc.scalar.copy(out=nodeo_sb[:, b * D:(b + 1) * D],
                                           in_=tp)

                if "store" in parts:
                    if layer == 0:
                        nc.sync.dma_start(
                            out=h_chunk.rearrange("(b p) f -> p b f", p=P),
                            in_=nodeh_sb.rearrange("p (b f) -> p b f", f=P),
                        )
                    else:
                        nc.sync.dma_start(
                            out=out_dr.ap().rearrange("(b p) f -> p b f", p=P),
                            in_=nodeo_sb.rearrange("p (b f) -> p b f", f=D),
                        )
                if layer == 0 and "collective" in parts:
                    if one_core:
                        nc.sync.dma_start(out=h_full[0:NP, :], in_=h_chunk)
                    else:
                        nc.gpsimd.collective_compute(
                            "AllGather",
                            mybir.AluOpType.bypass,
                            replica_groups=[list(range(NC))],
                            ins=[h_chunk.opt()],
                            outs=[h_full.opt()],
                        )

    nc.compile()
    return nc


def make_in_maps(meta, x, W_l1, b_l1, W_r1, W_l2, b_l2, W_r2):
    cfg = meta.cfg
    x = np.ascontiguousarray(np.asarray(x, dtype=np.float32))
    xp = np.zeros((cfg.N, P), BF)
    xp[:, :D] = x.astype(BF)
    iota = np.tile(np.arange(P, dtype=np.float32), (P, 1)).astype(BF)
    ident = np.eye(D, dtype=np.float32)
    common = {
        "xp": xp,
        "wl1t": np.ascontiguousarray(np.asarray(W_l1, np.float32).T),
        "wr1t": np.ascontiguousarray(np.asarray(W_r1, np.float32).T),
        "wl2t": np.ascontiguousarray(np.asarray(W_l2, np.float32).T),
        "wr2t": np.ascontiguousarray(np.asarray(W_r2, np.float32).T),
        "b1": np.asarray(b_l1, np.float32).reshape(D, 1).copy(),
        "b2": np.asarray(b_l2, np.float32).reshape(D, 1).copy(),
        "iota": iota,
        "ident": ident,
    }
    in_maps = []
    for k in range(cfg.n_cores):
        xo = x[k * cfg.n_own:(k + 1) * cfg.n_own]
        xoT = np.zeros((D, cfg.n_own_pad), np.float32)
        xoT[:, :cfg.n_own] = xo.T
        in_maps.append(dict(common, xoT=xoT, idx=meta.idx[k],
                            dstf=meta.dstf[k], invb=meta.invb[k]))
    return in_maps


_CACHE = {}
_LAST_RES = None


def kernel(x, edge_index, W_l1, b_l1, W_r1, W_l2, b_l2, W_r2):
    edge_index = np.asarray(edge_index)
    x = np.asarray(x)
    cfg = Cfg(x.shape[0])
    key = hash(edge_index.tobytes())
    if key in _CACHE:
        meta, nc = _CACHE[key]
    else:
        meta = preprocess(edge_index, cfg)
        nc = build_program(meta)
        _CACHE[key] = (meta, nc)
    in_maps = make_in_maps(meta, x, W_l1, b_l1, W_r1, W_l2, b_l2, W_r2)
    res = run_bass_kernel_spmd(nc, in_maps, core_ids=list(range(cfg.n_cores)))
    global _LAST_RES
    _LAST_RES = res
    out = np.concatenate(
        [res.results[k]["out"][:cfg.n_own] for k in range(cfg.n_cores)], axis=0
    )
    return out.astype(np.float32)



# revision 4
# speedup vs baseline: 1.8262x; 1.8262x over previous
"""2-layer GraphSAGE (mean aggregation) on 8 trn2 NeuronCores via Bass/Tile.

Strategy (matches the sharding hint):
  - Nodes are row-sharded across the 8 cores (6250 rows each); edges are
    partitioned by destination core.
  - Per core, edges are grouped by 128-node destination block.  Messages
    x[src] are fetched with InstDMAGatherAnt (edge-major tiles of 128) from a
    bf16 256B-padded copy of the features, and the segment-sum is computed as
    a one-hot matmul on the tensor engine (bf16, 1 cycle/row):
        aggT[64f, 128d] += msgs[128e, 64f].T @ onehot[128e, 128d]
    where onehot[e, d] = (d == dst_local[e]) is built on the vector engine
    from a broadcast iota with one tensor_scalar(is_equal) op per tile.
    The exact f32 1/deg scaling is applied at PSUM->SBUF copy time via a
    host-built [64, n] broadcast table (elementwise mult on DVE).
  - The 64x64 weights are replicated; the dense phase runs feature-major in
    f32 on rotating [64, 512] group buffers.
  - h = tanh(layer1) chunks (bf16-padded) are AllGathered between layers.
  - dma_gather indices are int16, so each gather call reads one of two row
    regions of the source, split at a core boundary so lo/hi membership is
    identical for the x-space and the padded h-space.
"""

import numpy as np
import ml_dtypes

import concourse.bacc as bacc
import concourse.mybir as mybir
import concourse.tile as tile
from concourse.bass_utils import run_bass_kernel_spmd

P = 128
D = 64
F32 = mybir.dt.float32
BF16 = mybir.dt.bfloat16
I16 = mybir.dt.int16
BF = ml_dtypes.bfloat16


NQ = 4  # SWDGE queues (ucode max); gather calls round-robin across them


class Cfg:
    def __init__(self, N, n_cores=8, chunk=64, msgs_bufs=4):
        assert N % n_cores == 0
        self.N = N
        self.n_cores = n_cores
        self.n_own = N // n_cores
        self.nblk = -(-self.n_own // P)
        self.n_own_pad = self.nblk * P
        self.n_pad_all = self.n_own_pad * n_cores
        # lo/hi split at a core boundary so that edge region membership is
        # identical for x-space (N rows) and padded h-space (n_pad_all rows).
        c = n_cores // 2
        while self.N - c * self.n_own > 32768 or self.n_pad_all - c * self.n_own_pad > 32768:
            c += 1
        assert c * self.n_own <= 32768 and c * self.n_own_pad <= 32768
        self.split_core = c
        self.split = c * self.n_own
        self.split_pad = c * self.n_own_pad
        self.chunk = chunk
        self.msgs_bufs = msgs_bufs


class Meta:
    pass


def _wrap16(v):
    """slot i -> [i % 16, i // 16] layout used by dma_gather idx tables."""
    assert v.shape[0] % 16 == 0
    return np.ascontiguousarray(v.reshape(-1, 16).T)


def preprocess(edge_index, cfg):
    """Partition/group edges; build per-core gather index + onehot tables."""
    src = np.asarray(edge_index[0], dtype=np.int64)
    dst = np.asarray(edge_index[1], dtype=np.int64)
    E = src.shape[0]
    NC, NBLK = cfg.n_cores, cfg.nblk

    cnt = np.bincount(dst, minlength=cfg.N).astype(np.float32)
    inv = (1.0 / np.maximum(cnt, 1.0)).astype(np.float32)

    core = dst // cfg.n_own
    dstl = dst - core * cfg.n_own
    blk = dstl // P
    inb = dstl - blk * P
    region = (src >= cfg.split).astype(np.int64)

    key = ((core * NBLK) + blk) * 2 + region
    ngroups = NC * NBLK * 2
    gcnt = np.bincount(key, minlength=ngroups).reshape(NC, NBLK, 2)
    # uniform (max over cores) tile counts per (block, region)
    TL = np.maximum(1, -(-gcnt[:, :, 0].max(axis=0) // P))
    TH = np.maximum(1, -(-gcnt[:, :, 1].max(axis=0) // P))
    lo_off = np.concatenate([[0], np.cumsum(TL)])
    hi_off = np.concatenate([[0], np.cumsum(TH)])
    TLT, THT = int(lo_off[-1]), int(hi_off[-1])
    T_ALL = TLT + THT

    # rank of each edge within its (core, blk, region) group
    order = np.argsort(key, kind="stable")
    gstart = np.concatenate([[0], np.cumsum(np.bincount(key, minlength=ngroups))])[:-1]
    rank = np.empty(E, dtype=np.int64)
    rank[order] = np.arange(E) - gstart[key[order]]

    # slot within region (tiles of 128)
    reg_base = np.where(region == 0, lo_off[blk], hi_off[blk])
    slot = reg_base * P + rank

    # h-space (padded) position of each source node
    pos = (src // cfg.n_own) * cfg.n_own_pad + (src % cfg.n_own)

    meta = Meta()
    meta.cfg = cfg
    meta.TL, meta.TH = TL, TH
    meta.TLT, meta.THT, meta.T_ALL = TLT, THT, T_ALL
    meta.block_tiles = [
        list(range(int(lo_off[b]), int(lo_off[b + 1])))
        + [TLT + t for t in range(int(hi_off[b]), int(hi_off[b + 1]))]
        for b in range(NBLK)
    ]

    # per-core tables
    meta.idx = []   # [128, 8*(TLT+THT)*2] int16 : l1lo | l1hi | l2lo | l2hi
    meta.dstf = []  # [128, T_ALL] f32
    meta.invb = []  # [64, n_own_pad] f32 : 1/deg broadcast down 64 partitions
    for k in range(NC):
        m = core == k
        sl = slot[m]
        rg = region[m]
        s_lo, s_hi = sl[rg == 0], sl[rg == 1]
        i1lo = np.zeros(TLT * P, np.int16)
        i1hi = np.zeros(THT * P, np.int16)
        i2lo = np.zeros(TLT * P, np.int16)
        i2hi = np.zeros(THT * P, np.int16)
        i1lo[s_lo] = src[m][rg == 0]
        i1hi[s_hi] = src[m][rg == 1] - cfg.split
        i2lo[s_lo] = pos[m][rg == 0]
        i2hi[s_hi] = pos[m][rg == 1] - cfg.split_pad
        w = np.concatenate(
            [_wrap16(a) for a in (i1lo, i1hi, i2lo, i2hi)], axis=1)
        # the gather ucode reads each Q7 core's idx stripe from its own
        # 16-partition group -> replicate 8x down the partition axis
        meta.idx.append(np.ascontiguousarray(np.tile(w, (8, 1))))

        df = np.full(T_ALL * P, -1.0, np.float32)
        gs = np.where(rg == 0, 0, TLT * P) + sl
        df[gs] = inb[m].astype(np.float32)
        meta.dstf.append(np.ascontiguousarray(df.reshape(T_ALL, P).T))

        iv = np.ones(cfg.n_own_pad, np.float32)
        iv[:cfg.n_own] = inv[k * cfg.n_own:(k + 1) * cfg.n_own]
        meta.invb.append(np.ascontiguousarray(np.tile(iv, (D, 1))))

    meta.idx_off = [0, TLT * 8, (TLT + THT) * 8, (2 * TLT + THT) * 8]

    # gather calls: (region, t0, ntiles, first_block), interleaved by the
    # first destination block each chunk serves.
    def chunks(T_total, offs):
        out = []
        t0 = 0
        while t0 < T_total:
            nt = min(cfg.chunk, T_total - t0)
            fb = int(np.searchsorted(offs, t0, side="right") - 1)
            out.append((t0, nt, fb))
            t0 += nt
        return out

    calls = [(0, t0, nt, fb) for (t0, nt, fb) in chunks(TLT, lo_off)]
    calls += [(1, t0, nt, fb) for (t0, nt, fb) in chunks(THT, hi_off)]
    calls.sort(key=lambda c: (c[3], c[0]))
    meta.calls = calls
    return meta


GCOL = 512  # dense-phase group width (one PSUM bank)


def build_program(meta, one_core=False,
                  parts=("gather", "agg", "dense", "store", "collective"),
                  reps=1):
    cfg = meta.cfg
    NC, NBLK = cfg.n_cores, cfg.nblk
    NP = cfg.n_own_pad
    BPG = GCOL // P  # blocks per dense group
    nc = bacc.Bacc(
        "TRN2", target_bir_lowering=False, debug=False,
        num_devices=1 if one_core else NC,
        num_swdge_queues=NQ,
    )

    xp_dr = nc.dram_tensor("xp", [cfg.N, P], BF16, kind="ExternalInput")
    xoT_dr = nc.dram_tensor("xoT", [D, NP], F32, kind="ExternalInput")
    idx_dr = nc.dram_tensor("idx", list(meta.idx[0].shape), I16, kind="ExternalInput")
    dstf_dr = nc.dram_tensor("dstf", [P, meta.T_ALL], F32, kind="ExternalInput")
    invb_dr = nc.dram_tensor("invb", [D, NP], F32, kind="ExternalInput")
    wl1_dr = nc.dram_tensor("wl1t", [D, D], F32, kind="ExternalInput")
    wr1_dr = nc.dram_tensor("wr1t", [D, D], F32, kind="ExternalInput")
    wl2_dr = nc.dram_tensor("wl2t", [D, D], F32, kind="ExternalInput")
    wr2_dr = nc.dram_tensor("wr2t", [D, D], F32, kind="ExternalInput")
    b1_dr = nc.dram_tensor("b1", [D, 1], F32, kind="ExternalInput")
    b2_dr = nc.dram_tensor("b2", [D, 1], F32, kind="ExternalInput")
    iota_dr = nc.dram_tensor("iota", [P, P], BF16, kind="ExternalInput")
    id_dr = nc.dram_tensor("ident", [D, D], F32, kind="ExternalInput")
    out_dr = nc.dram_tensor("out", [NP, D], F32, kind="ExternalOutput")

    with tile.TileContext(nc) as tc:
        with (
            tc.tile_pool(name="const", bufs=1) as cpool,
            tc.tile_pool(name="big", bufs=1) as bpool,
            tc.tile_pool(name="msgs", bufs=cfg.msgs_bufs) as mpool,
            tc.tile_pool(name="idxp", bufs=4) as ipool,
            tc.tile_pool(name="ohp", bufs=12) as ohpool,
            tc.tile_pool(name="grp", bufs=2) as gpool,
            tc.tile_pool(name="psA", bufs=4, space="PSUM") as psA,
            tc.tile_pool(name="psZ", bufs=2, space="PSUM") as psZ,
            tc.tile_pool(name="psT", bufs=2, space="PSUM") as psT,
            tc.tile_pool(name="dram", bufs=1, space="DRAM") as dpool,
        ):
            def load(pool, dr, shape, name, dt=F32, tag=""):
                t = pool.tile(shape, dt, name=name, tag=tag or name)
                nc.sync.dma_start(out=t, in_=dr.ap())
                return t

            iota_sb = load(cpool, iota_dr, [P, P], "iota_sb", dt=BF16)
            ident_sb = load(cpool, id_dr, [D, D], "ident_sb")
            wl1_sb = load(cpool, wl1_dr, [D, D], "wl1_sb")
            wr1_sb = load(cpool, wr1_dr, [D, D], "wr1_sb")
            wl2_sb = load(cpool, wl2_dr, [D, D], "wl2_sb")
            wr2_sb = load(cpool, wr2_dr, [D, D], "wr2_sb")
            b1_sb = load(cpool, b1_dr, [D, 1], "b1_sb")
            b2_sb = load(cpool, b2_dr, [D, 1], "b2_sb")
            dstf_sb = load(bpool, dstf_dr, [P, meta.T_ALL], "dstf_sb")
            invb_sb = load(bpool, invb_dr, [D, NP], "invb_sb")
            xoT_sb = load(bpool, xoT_dr, [D, NP], "xoT_sb")
            hT_sb = bpool.tile([D, NP], F32, name="hT_sb")
            nodeh_sb = bpool.tile([P, NBLK * P], BF16, name="nodeh_sb")
            nodeo_sb = bpool.tile([P, NBLK * D], F32, name="nodeo_sb")
            # zero the bf16 pad columns once (cols [b*128+64, b*128+128))
            nc.vector.memset(nodeh_sb, 0.0)

            for rep in range(reps):
              h_chunk = dpool.tile([NP, P], BF16, name=f"h_chunk_{rep}", tag=f"hc{rep}")
              h_full = dpool.tile([cfg.n_pad_all, P], BF16, name=f"h_full_{rep}",
                                  tag=f"hf{rep}", addr_space="Shared")
              for layer in range(2):
                if layer == 0:
                    src_lo = xp_dr.ap()[0:cfg.split, :]
                    src_hi = xp_dr.ap()[cfg.split:cfg.N, :]
                    off_lo, off_hi = meta.idx_off[0], meta.idx_off[1]
                else:
                    src_lo = h_full[0:cfg.split_pad, :]
                    src_hi = h_full[cfg.split_pad:cfg.n_pad_all, :]
                    off_lo, off_hi = meta.idx_off[2], meta.idx_off[3]

                # ---- gather messages (bf16, 256B rows) ----
                tsrc = {}
                for ci, (rg, t0, ntile, _fb) in enumerate(meta.calls):
                    mt = mpool.tile([P, cfg.chunk, P], BF16, tag="msgs",
                                    name=f"m_{layer}_{ci}")
                    if "gather" in parts:
                        it = ipool.tile([P, cfg.chunk * 8], I16, tag="idx",
                                        name=f"i_{layer}_{ci}")
                        cols = ntile * 8
                        coff = (off_lo if rg == 0 else off_hi) + t0 * 8
                        nc.sync.dma_start(out=it[:, :cols],
                                          in_=idx_dr.ap()[:, coff:coff + cols])
                        nc.gpsimd.dma_gather(
                            mt[:, :ntile, :],
                            src_lo if rg == 0 else src_hi,
                            it[:, :cols],
                            num_idxs=ntile * P,
                            num_idxs_reg=ntile * P,
                            elem_size=P,
                            single_packet=False,
                            queue_num=ci % NQ,
                        )
                    base = t0 if rg == 0 else meta.TLT + t0
                    for j in range(ntile):
                        tsrc[base + j] = (mt, j)

                # ---- blocks: onehot matmul segment-sum + dense per group ----
                if layer == 0:
                    wl_sb, wr_sb, bb_sb = wl1_sb, wr1_sb, b1_sb
                    own_sb = xoT_sb
                    func = mybir.ActivationFunctionType.Tanh
                else:
                    wl_sb, wr_sb, bb_sb = wl2_sb, wr2_sb, b2_sb
                    own_sb = hT_sb
                    func = mybir.ActivationFunctionType.Identity

                ngrp = -(-NBLK // BPG)
                for g in range(ngrp if "agg" in parts else 0):
                    b0 = g * BPG
                    nb = min(BPG, NBLK - b0)
                    w = nb * P
                    aggT = gpool.tile([D, GCOL], F32, tag="aggT",
                                      name=f"agg_{rep}_{layer}_{g}")
                    for bi in range(nb):
                        b = b0 + bi
                        ps = psA.tile([D, P], F32, tag="agg", name=f"ps_{layer}_{b}")
                        gts = meta.block_tiles[b]
                        for j, gt in enumerate(gts):
                            oh = ohpool.tile([P, P], BF16, tag="oh",
                                             name=f"oh_{layer}_{b}_{j}")
                            nc.vector.tensor_scalar(
                                out=oh, in0=iota_sb,
                                scalar1=dstf_sb[:, gt:gt + 1],
                                scalar2=None,
                                op0=mybir.AluOpType.is_equal,
                            )
                            mt, lt = tsrc[gt]
                            nc.tensor.matmul(
                                ps, lhsT=mt[:, lt, 0:D], rhs=oh,
                                start=(j == 0), stop=(j == len(gts) - 1),
                            )
                        # exact mean scaling: psum * (1/deg) broadcast table
                        nc.vector.tensor_tensor(
                            out=aggT[:, bi * P:(bi + 1) * P], in0=ps,
                            in1=invb_sb[:, b * P:(b + 1) * P],
                            op=mybir.AluOpType.mult,
                        )
                    if "dense" not in parts:
                        continue
                    zp = psZ.tile([D, GCOL], F32, tag="z", name=f"z_{layer}_{g}")
                    nc.tensor.matmul(zp[:, :w], lhsT=wl_sb, rhs=aggT[:, :w],
                                     start=True, stop=False)
                    nc.tensor.matmul(zp[:, :w], lhsT=wr_sb,
                                     rhs=own_sb[:, b0 * P:b0 * P + w],
                                     start=False, stop=True)
                    if layer == 0:
                        outT = hT_sb
                        nc.scalar.activation(out=hT_sb[:, b0 * P:b0 * P + w],
                                             in_=zp[:, :w], func=func,
                                             bias=bb_sb[:, 0:1], scale=1.0)
                    else:
                        outT = gpool.tile([D, GCOL], F32, tag="outT",
                                          name=f"oT_{rep}_{g}")
                        nc.scalar.activation(out=outT[:, :w], in_=zp[:, :w],
                                             func=func, bias=bb_sb[:, 0:1],
                                             scale=1.0)
                    if "store" not in parts:
                        continue
                    for bi in range(nb):
                        b = b0 + bi
                        tp = psT.tile([P, D], F32, tag="tr", name=f"tp_{layer}_{b}")
                        sl = (slice(b * P, b * P + P) if layer == 0
                              else slice(bi * P, bi * P + P))
                        nc.tensor.transpose(out=tp, in_=outT[:, sl],
                                            identity=ident_sb)
                        if layer == 0:
                            # bf16 padded node-major h rows
                            nc.scalar.copy(out=nodeh_sb[:, b * P:b * P + D],
                                           in_=tp)
                        else:
                            nc.scalar.copy(out=nodeo_sb[:, b * D:(b + 1) * D],
                                           in_=tp)

                if "store" in parts:
                    if layer == 0:
                        nc.sync.dma_start(
                            out=h_chunk.rearrange("(b p) f -> p b f", p=P),
                            in_=nodeh_sb.rearrange("p (b f) -> p b f", f=P),
                        )
                    else:
                        nc.sync.dma_start(
                            out=out_dr.ap().rearrange("(b p) f -> p b f", p=P),
                            in_=nodeo_sb.rearrange("p (b f) -> p b f", f=D),
                        )
                if layer == 0 and "collective" in parts:
                    if one_core:
                        nc.sync.dma_start(out=h_full[0:NP, :], in_=h_chunk)
                    else:
                        nc.gpsimd.collective_compute(
                            "AllGather",
                            mybir.AluOpType.bypass,
                            replica_groups=[list(range(NC))],
                            ins=[h_chunk.opt()],
                            outs=[h_full.opt()],
                        )

    nc.compile()
    return nc


def make_in_maps(meta, x, W_l1, b_l1, W_r1, W_l2, b_l2, W_r2):
    cfg = meta.cfg
    x = np.ascontiguousarray(np.asarray(x, dtype=np.float32))
    xp = np.zeros((cfg.N, P), BF)
    xp[:, :D] = x.astype(BF)
    iota = np.tile(np.arange(P, dtype=np.float32), (P, 1)).astype(BF)
    ident = np.eye(D, dtype=np.float32)
    common = {
        "xp": xp,
        "wl1t": np.ascontiguousarray(np.asarray(W_l1, np.float32).T),
        "wr1t": np.ascontiguousarray(np.asarray(W_r1, np.float32).T),
        "wl2t": np.ascontiguousarray(np.asarray(W_l2, np.float32).T),
        "wr2t": np.ascontiguousarray(np.asarray(W_r2, np.float32).T),
        "b1": np.asarray(b_l1, np.float32).reshape(D, 1).copy(),
        "b2": np.asarray(b_l2, np.float32).reshape(D, 1).copy(),
        "iota": iota,
        "ident": ident,
    }
    in_maps = []
    for k in range(cfg.n_cores):
        xo = x[k * cfg.n_own:(k + 1) * cfg.n_own]
        xoT = np.zeros((D, cfg.n_own_pad), np.float32)
        xoT[:, :cfg.n_own] = xo.T
        in_maps.append(dict(common, xoT=xoT, idx=meta.idx[k],
                            dstf=meta.dstf[k], invb=meta.invb[k]))
    return in_maps


_CACHE = {}
_LAST_RES = None


def kernel(x, edge_index, W_l1, b_l1, W_r1, W_l2, b_l2, W_r2):
    edge_index = np.asarray(edge_index)
    x = np.asarray(x)
    cfg = Cfg(x.shape[0])
    key = hash(edge_index.tobytes())
    if key in _CACHE:
        meta, nc = _CACHE[key]
    else:
        meta = preprocess(edge_index, cfg)
        nc = build_program(meta)
        _CACHE[key] = (meta, nc)
    in_maps = make_in_maps(meta, x, W_l1, b_l1, W_r1, W_l2, b_l2, W_r2)
    res = run_bass_kernel_spmd(nc, in_maps, core_ids=list(range(cfg.n_cores)))
    global _LAST_RES
    _LAST_RES = res
    out = np.concatenate(
        [res.results[k]["out"][:cfg.n_own] for k in range(cfg.n_cores)], axis=0
    )
    return out.astype(np.float32)



# revision 37
# speedup vs baseline: 4.0027x; 2.1918x over previous
"""2-layer GraphSAGE (mean aggregation) on 8 trn2 NeuronCores via Bass/Tile.

Strategy (matches the sharding hint):
  - Nodes are row-sharded across the 8 cores (6250 rows each); edges are
    partitioned by destination core.
  - Per core, edges are grouped by 128-node destination block.  Messages
    x[src] are fetched with InstDMAGatherAnt (edge-major tiles of 128) from a
    bf16 256B-padded copy of the features, and the segment-sum is computed as
    a one-hot matmul on the tensor engine (bf16, 1 cycle/row):
        aggT[64f, 128d] += msgs[128e, 64f].T @ onehot[128e, 128d]
    where onehot[e, d] = (d == dst_local[e]) is built on the vector engine
    from a broadcast iota with one tensor_scalar(is_equal) op per tile.
    The exact f32 1/deg scaling is applied at PSUM->SBUF copy time via a
    host-built [64, n] broadcast table (elementwise mult on DVE).
  - The 64x64 weights are replicated; the dense phase runs feature-major in
    f32 on rotating [64, 512] group buffers.
  - h = tanh(layer1) chunks (bf16-padded) are AllGathered between layers.
  - dma_gather indices are int16, so each gather call reads one of two row
    regions of the source, split at a core boundary so lo/hi membership is
    identical for the x-space and the padded h-space.
"""

import numpy as np
import ml_dtypes

import concourse.bacc as bacc
import concourse.mybir as mybir
import concourse.tile as tile
from concourse.bass_utils import run_bass_kernel_spmd

P = 128
D = 64
F32 = mybir.dt.float32
BF16 = mybir.dt.bfloat16
FP8 = mybir.dt.float8e4
I16 = mybir.dt.int16
BF = ml_dtypes.bfloat16
F8 = ml_dtypes.float8_e4m3


NQ = 4  # SWDGE queues (ucode max); gather calls round-robin across them


class Cfg:
    def __init__(self, N, n_cores=8, chunk=34, msgs_bufs=6,
                 rebalance=True, sort_src=True, single_packet=False,
                 extra_blocks=1):
        assert N % n_cores == 0
        self.N = N
        self.n_cores = n_cores
        self.n_own = N // n_cores
        self.nblk = -(-self.n_own // P) + (extra_blocks if rebalance else 0)
        self.n_own_pad = self.nblk * P
        self.n_pad_all = self.n_own_pad * n_cores
        # lo/hi split at a core boundary so that edge region membership is
        # identical for x-space (N rows) and padded h-space (n_pad_all rows).
        c = n_cores // 2
        while self.N - c * self.n_own > 32768 or self.n_pad_all - c * self.n_own_pad > 32768:
            c += 1
        assert c * self.n_own <= 32768 and c * self.n_own_pad <= 32768
        self.split_core = c
        self.split = c * self.n_own
        self.split_pad = c * self.n_own_pad
        self.chunk = chunk
        self.msgs_bufs = msgs_bufs
        self.rebalance = rebalance
        self.sort_src = sort_src
        self.single_packet = single_packet


class Meta:
    pass


def _wrap16(v):
    """slot i -> [i % 16, i // 16] layout used by dma_gather idx tables."""
    assert v.shape[0] % 16 == 0
    return np.ascontiguousarray(v.reshape(-1, 16).T)


def _pack_blocks(a, b, nbins, cap_rows, n_rows_total):
    """Greedy weighted-target LPT: assign items (weights a[i], b[i]) to nbins
    bins (row capacity cap_rows, last bin n_rows_total-(nbins-1)*cap_rows).
    Bins 0..nbins-2 aim just under 8*128 edges per region so their gather
    tiles stay at 8; the last bin absorbs the remainder.  Returns bin id per
    item."""
    n = a.shape[0]
    order = np.argsort(-(a + b), kind="stable")
    caps = np.full(nbins, cap_rows, np.int64)
    target = 8 * P - 8  # 1016: leave slack below the 8-tile ceiling
    tA = np.full(nbins, target, np.float64)
    tA[-1] = max(float(a.sum() - (nbins - 1) * target), 200.0)
    tB = np.full(nbins, target, np.float64)
    tB[-1] = max(float(b.sum() - (nbins - 1) * target), 200.0)
    binA = np.zeros(nbins, np.int64)
    binB = np.zeros(nbins, np.int64)
    binc = np.zeros(nbins, np.int64)
    bin_of = np.empty(n, np.int64)
    for i in order:
        av, bv = a[i], b[i]
        load = np.maximum((binA + av) / tA, (binB + bv) / tB)
        load[binc >= caps] = np.inf
        j = int(np.argmin(load))
        bin_of[i] = j
        binA[j] += av
        binB[j] += bv
        binc[j] += 1
    return bin_of


def preprocess(edge_index, cfg):
    """Partition/group edges; build per-core gather index + onehot tables."""
    src = np.asarray(edge_index[0], dtype=np.int64)
    dst = np.asarray(edge_index[1], dtype=np.int64)
    E = src.shape[0]
    NC, NBLK = cfg.n_cores, cfg.nblk

    cnt = np.bincount(dst, minlength=cfg.N).astype(np.float32)
    inv = (1.0 / np.maximum(cnt, 1.0)).astype(np.float32)

    core = dst // cfg.n_own
    dstl = dst - core * cfg.n_own
    region = (src >= cfg.split).astype(np.int64)

    if cfg.rebalance:
        # Per-core permutation of destination rows so that per-(block,
        # region) edge counts stay near-uniform -> fewer padded gather tiles.
        deg = np.zeros((cfg.N, 2), np.int64)
        np.add.at(deg, (dst, region), 1)
        perms = []
        for k in range(NC):
            dl = deg[k * cfg.n_own:(k + 1) * cfg.n_own]
            bin_of = _pack_blocks(dl[:, 0], dl[:, 1], NBLK, P, cfg.n_own)
            perm = np.empty(cfg.n_own, np.int64)
            row = 0
            for bb in range(NBLK):
                members = np.nonzero(bin_of == bb)[0]
                perm[members] = bb * P + np.arange(len(members))
            perms.append(perm)
        meta_perm = perms
        pa = np.concatenate(perms)  # indexed by global node id
        dstl = pa[dst]
    else:
        meta_perm = [np.arange(cfg.n_own) for _ in range(NC)]

    blk = dstl // P
    inb = dstl - blk * P

    key = ((core * NBLK) + blk) * 2 + region
    ngroups = NC * NBLK * 2
    gcnt = np.bincount(key, minlength=ngroups).reshape(NC, NBLK, 2)
    # uniform (max over cores) tile counts per (block, region)
    TL = np.maximum(1, -(-gcnt[:, :, 0].max(axis=0) // P))
    TH = np.maximum(1, -(-gcnt[:, :, 1].max(axis=0) // P))
    lo_off = np.concatenate([[0], np.cumsum(TL)])
    hi_off = np.concatenate([[0], np.cumsum(TH)])
    TLT, THT = int(lo_off[-1]), int(hi_off[-1])
    T_ALL = TLT + THT

    # rank of each edge within its (core, blk, region) group; sort_src makes
    # slot order follow ascending source row for HBM locality
    if cfg.sort_src:
        order = np.lexsort((src, key))
    else:
        order = np.argsort(key, kind="stable")
    gstart = np.concatenate([[0], np.cumsum(np.bincount(key, minlength=ngroups))])[:-1]
    rank = np.empty(E, dtype=np.int64)
    rank[order] = np.arange(E) - gstart[key[order]]

    # slot within region (tiles of 128)
    reg_base = np.where(region == 0, lo_off[blk], hi_off[blk])
    slot = reg_base * P + rank

    # h-space (padded) position of each source node (via the dest perm of the
    # source's owner core)
    pa_all = np.concatenate(meta_perm)
    pos = (src // cfg.n_own) * cfg.n_own_pad + pa_all[src]

    meta = Meta()
    meta.cfg = cfg
    meta.perm = meta_perm
    meta.TL, meta.TH = TL, TH
    meta.TLT, meta.THT, meta.T_ALL = TLT, THT, T_ALL
    meta.block_tiles = [
        list(range(int(lo_off[b]), int(lo_off[b + 1])))
        + [TLT + t for t in range(int(hi_off[b]), int(hi_off[b + 1]))]
        for b in range(NBLK)
    ]

    # per-core tables
    meta.idx = []   # [128, 8*(TLT+THT)*2] int16 : l1lo | l1hi | l2lo | l2hi
    meta.dstf = []  # [128, T_ALL] f32
    meta.oh8 = []   # [128, T_ALL*128] fp8 one-hot tiles
    meta.invb = []  # [64, n_own_pad] f32 : 1/deg broadcast down 64 partitions
    for k in range(NC):
        m = core == k
        sl = slot[m]
        rg = region[m]
        s_lo, s_hi = sl[rg == 0], sl[rg == 1]
        i1lo = np.zeros(TLT * P, np.int16)
        i1hi = np.zeros(THT * P, np.int16)
        i2lo = np.zeros(TLT * P, np.int16)
        i2hi = np.zeros(THT * P, np.int16)
        i1lo[s_lo] = src[m][rg == 0]
        i1hi[s_hi] = src[m][rg == 1] - cfg.split
        i2lo[s_lo] = pos[m][rg == 0]
        i2hi[s_hi] = pos[m][rg == 1] - cfg.split_pad
        w = np.concatenate(
            [_wrap16(a) for a in (i1lo, i1hi, i2lo, i2hi)], axis=1)
        # the gather ucode reads each Q7 core's idx stripe from its own
        # 16-partition group -> replicate 8x down the partition axis
        meta.idx.append(np.ascontiguousarray(np.tile(w, (8, 1))))

        df = np.full(T_ALL * P, -1.0, np.float32)
        gs = np.where(rg == 0, 0, TLT * P) + sl
        df[gs] = inb[m].astype(np.float32)
        meta.dstf.append(np.ascontiguousarray(df.reshape(T_ALL, P).T))
        # precomputed one-hot tiles (fp8: 0/1 exact): [p, t*128+d]
        dfr = df.reshape(T_ALL, P)
        oh = dfr[:, :, None] == np.arange(P, dtype=np.float32)[None, None, :]
        meta.oh8.append(np.ascontiguousarray(
            oh.transpose(1, 0, 2).reshape(P, T_ALL * P).astype(F8)))

        iv = np.ones(cfg.n_own_pad, np.float32)
        iv[meta_perm[k]] = inv[k * cfg.n_own:(k + 1) * cfg.n_own]
        meta.invb.append(np.ascontiguousarray(np.tile(iv, (D, 1)).astype(BF)))

    meta.idx_off = [0, TLT * 8, (TLT + THT) * 8, (2 * TLT + THT) * 8]

    # gather calls: (region, t0, ntiles, first_block), interleaved by the
    # first destination block each chunk serves.  The TOTAL number of calls
    # per layer must be a multiple of 8: Tile assigns DMA-completion sems
    # round-robin over 8 DMASW lanes while queues rotate mod NQ — a call
    # count ==0 mod 8 keeps every sem pinned to one queue (same-sem calls
    # then always share a queue and complete FIFO; otherwise two queues can
    # bump one sem out of order and a consumer wait can fire early).
    def chunks(T_total, ncalls, offs):
        assert T_total >= ncalls
        base, rem = divmod(T_total, ncalls)
        out = []
        t0 = 0
        for i in range(ncalls):
            nt = base + (1 if i < rem else 0)
            fb = int(np.searchsorted(offs, t0, side="right") - 1)
            out.append((t0, nt, fb))
            t0 += nt
        assert t0 == T_total
        return out

    ncalls_lo = -(-TLT // cfg.chunk)
    ncalls_hi = -(-THT // cfg.chunk)
    pad = -(ncalls_lo + ncalls_hi) % 8
    ncalls_lo += pad // 2 + pad % 2
    ncalls_hi += pad // 2
    calls = [(0, t0, nt, fb) for (t0, nt, fb) in chunks(TLT, ncalls_lo, lo_off)]
    calls += [(1, t0, nt, fb) for (t0, nt, fb) in chunks(THT, ncalls_hi, hi_off)]
    calls.sort(key=lambda c: (c[3], c[0]))
    meta.calls = calls
    return meta


GCOL = 512  # dense-phase group width (one PSUM bank)


def build_program(meta, one_core=False,
                  parts=("gather", "agg", "dense", "store", "collective"),
                  reps=1):
    cfg = meta.cfg
    NC, NBLK = cfg.n_cores, cfg.nblk
    NP = cfg.n_own_pad
    BPG = GCOL // P  # blocks per dense group
    nc = bacc.Bacc(
        "TRN2", target_bir_lowering=False, debug=False,
        num_devices=1 if one_core else NC,
        num_swdge_queues=NQ,
    )

    xp_dr = nc.dram_tensor("xp", [cfg.N, P], BF16, kind="ExternalInput")
    xoT_dr = nc.dram_tensor("xoT", [D, NP], F32, kind="ExternalInput")
    idx_dr = nc.dram_tensor("idx", list(meta.idx[0].shape), I16, kind="ExternalInput")
    oh8_dr = nc.dram_tensor("oh8", [P, meta.T_ALL * P], FP8, kind="ExternalInput")
    invb_dr = nc.dram_tensor("invb", [D, NP], BF16, kind="ExternalInput")
    wl1_dr = nc.dram_tensor("wl1t", [D, D], F32, kind="ExternalInput")
    wr1_dr = nc.dram_tensor("wr1t", [D, D], F32, kind="ExternalInput")
    wl2_dr = nc.dram_tensor("wl2t", [D, D], F32, kind="ExternalInput")
    wr2_dr = nc.dram_tensor("wr2t", [D, D], F32, kind="ExternalInput")
    b1_dr = nc.dram_tensor("b1", [D, 1], F32, kind="ExternalInput")
    b2_dr = nc.dram_tensor("b2", [D, 1], F32, kind="ExternalInput")
    id_dr = nc.dram_tensor("ident", [D, D], F32, kind="ExternalInput")
    out_dr = nc.dram_tensor("out", [NP, D], F32, kind="ExternalOutput")

    with tile.TileContext(nc) as tc:
        with (
            tc.tile_pool(name="const", bufs=1) as cpool,
            tc.tile_pool(name="big", bufs=1) as bpool,
            tc.tile_pool(name="msgs", bufs=cfg.msgs_bufs) as mpool,
            tc.tile_pool(name="idxp", bufs=4) as ipool,
            tc.tile_pool(name="ohp", bufs=cfg.msgs_bufs) as ohpool,
            tc.tile_pool(name="grp", bufs=2) as gpool,
            tc.tile_pool(name="psA", bufs=4, space="PSUM") as psA,
            tc.tile_pool(name="psZ", bufs=2, space="PSUM") as psZ,
            tc.tile_pool(name="psT", bufs=2, space="PSUM") as psT,
            tc.tile_pool(name="dram", bufs=1, space="DRAM") as dpool,
        ):
            def load(pool, dr, shape, name, dt=F32, tag=""):
                t = pool.tile(shape, dt, name=name, tag=tag or name)
                nc.sync.dma_start(out=t, in_=dr.ap())
                return t

            ident_sb = load(cpool, id_dr, [D, D], "ident_sb")
            wl1_sb = load(cpool, wl1_dr, [D, D], "wl1_sb")
            wr1_sb = load(cpool, wr1_dr, [D, D], "wr1_sb")
            wl2_sb = load(cpool, wl2_dr, [D, D], "wl2_sb")
            wr2_sb = load(cpool, wr2_dr, [D, D], "wr2_sb")
            b1_sb = load(cpool, b1_dr, [D, 1], "b1_sb")
            b2_sb = load(cpool, b2_dr, [D, 1], "b2_sb")
            invb_sb = load(bpool, invb_dr, [D, NP], "invb_sb", dt=BF16)
            xoT_sb = load(bpool, xoT_dr, [D, NP], "xoT_sb")
            # double-buffered across reps so rep r+1's layer 1 (which writes
            # hT) can overlap rep r's layer 2 (which reads it)
            hT_bufs = [bpool.tile([D, NP], F32, name=f"hT_sb{i}", tag=f"hT{i}")
                       for i in range(min(2, reps))]

            for rep in range(reps):
              hT_sb = hT_bufs[rep % len(hT_bufs)]
              h_chunk = dpool.tile([NP, P], BF16, name=f"h_chunk_{rep}", tag=f"hc{rep}")
              h_full = dpool.tile([cfg.n_pad_all, P], BF16, name=f"h_full_{rep}",
                                  tag=f"hf{rep}", addr_space="Shared")
              for layer in range(2):
                if layer == 0:
                    src_lo = xp_dr.ap()[0:cfg.split, :]
                    src_hi = xp_dr.ap()[cfg.split:cfg.N, :]
                    off_lo, off_hi = meta.idx_off[0], meta.idx_off[1]
                else:
                    src_lo = h_full[0:cfg.split_pad, :]
                    src_hi = h_full[cfg.split_pad:cfg.n_pad_all, :]
                    off_lo, off_hi = meta.idx_off[2], meta.idx_off[3]

                # ---- gather messages (bf16, 256B rows) + stream one-hots ----
                tsrc = {}
                for ci, (rg, t0, ntile, _fb) in enumerate(meta.calls):
                    mt = mpool.tile([P, cfg.chunk, P], BF16, tag="msgs",
                                    name=f"m_{layer}_{ci}")
                    ohl = ohpool.tile([P, cfg.chunk * P], FP8, tag="oh",
                                      name=f"oh_{layer}_{ci}")
                    base = t0 if rg == 0 else meta.TLT + t0
                    if "agg" in parts:
                        nc.sync.dma_start(
                            out=ohl[:, :ntile * P],
                            in_=oh8_dr.ap()[:, base * P:(base + ntile) * P])
                    if "gather" in parts:
                        it = ipool.tile([P, cfg.chunk * 8], I16, tag="idx",
                                        name=f"i_{layer}_{ci}")
                        cols = ntile * 8
                        coff = (off_lo if rg == 0 else off_hi) + t0 * 8
                        nc.sync.dma_start(out=it[:, :cols],
                                          in_=idx_dr.ap()[:, coff:coff + cols])
                        nc.gpsimd.dma_gather(
                            mt[:, :ntile, :],
                            src_lo if rg == 0 else src_hi,
                            it[:, :cols],
                            num_idxs=ntile * P,
                            num_idxs_reg=ntile * P,
                            elem_size=P,
                            single_packet=cfg.single_packet,
                            queue_num=ci % NQ,
                        )
                    for j in range(ntile):
                        tsrc[base + j] = (mt, j, ohl)

                # ---- blocks: onehot matmul segment-sum + dense per group ----
                if layer == 0:
                    wl_sb, wr_sb, bb_sb = wl1_sb, wr1_sb, b1_sb
                    own_sb = xoT_sb
                    func = mybir.ActivationFunctionType.Tanh
                else:
                    wl_sb, wr_sb, bb_sb = wl2_sb, wr2_sb, b2_sb
                    own_sb = hT_sb
                    func = mybir.ActivationFunctionType.Identity

                ngrp = -(-NBLK // BPG)
                for g in range(ngrp if "agg" in parts else 0):
                    b0 = g * BPG
                    nb = min(BPG, NBLK - b0)
                    w = nb * P
                    aggT = gpool.tile([D, GCOL], F32, tag="aggT",
                                      name=f"agg_{rep}_{layer}_{g}")
                    for bi in range(nb):
                        b = b0 + bi
                        ps = psA.tile([D, P], F32, tag="agg", name=f"ps_{layer}_{b}")
                        gts = meta.block_tiles[b]
                        for j, gt in enumerate(gts):
                            mt, lt, ohl = tsrc[gt]
                            nc.tensor.matmul(
                                ps, lhsT=mt[:, lt, 0:D],
                                rhs=ohl[:, lt * P:(lt + 1) * P],
                                start=(j == 0), stop=(j == len(gts) - 1),
                            )
                        # exact mean scaling: psum * (1/deg) broadcast table
                        nc.vector.tensor_tensor(
                            out=aggT[:, bi * P:(bi + 1) * P], in0=ps,
                            in1=invb_sb[:, b * P:(b + 1) * P],
                            op=mybir.AluOpType.mult,
                        )
                    if "dense" not in parts:
                        continue
                    zp = psZ.tile([D, GCOL], F32, tag="z", name=f"z_{layer}_{g}")
                    nc.tensor.matmul(zp[:, :w], lhsT=wl_sb, rhs=aggT[:, :w],
                                     start=True, stop=False)
                    nc.tensor.matmul(zp[:, :w], lhsT=wr_sb,
                                     rhs=own_sb[:, b0 * P:b0 * P + w],
                                     start=False, stop=True)
                    if layer == 0:
                        outT = hT_sb
                        nc.scalar.activation(out=hT_sb[:, b0 * P:b0 * P + w],
                                             in_=zp[:, :w], func=func,
                                             bias=bb_sb[:, 0:1], scale=1.0)
                    else:
                        outT = gpool.tile([D, GCOL], F32, tag="outT",
                                          name=f"oT_{rep}_{g}")
                        nc.scalar.activation(out=outT[:, :w], in_=zp[:, :w],
                                             func=func, bias=bb_sb[:, 0:1],
                                             scale=1.0)
                    if "store" not in parts:
                        continue
                    # per-group node-major staging + store (h pad columns are
                    # never consumed downstream, so they may hold garbage)
                    if layer == 0:
                        ndg = gpool.tile([P, BPG * P], BF16, tag="ndh",
                                         name=f"ndh_{rep}_{g}")
                    else:
                        ndg = gpool.tile([P, BPG * D], F32, tag="ndo",
                                         name=f"ndo_{rep}_{g}")
                    for bi in range(nb):
                        b = b0 + bi
                        tp = psT.tile([P, D], F32, tag="tr", name=f"tp_{layer}_{b}")
                        sl = (slice(b * P, b * P + P) if layer == 0
                              else slice(bi * P, bi * P + P))
                        nc.tensor.transpose(out=tp, in_=outT[:, sl],
                                            identity=ident_sb)
                        if layer == 0:
                            nc.scalar.copy(out=ndg[:, bi * P:bi * P + D],
                                           in_=tp)
                        else:
                            nc.scalar.copy(out=ndg[:, bi * D:(bi + 1) * D],
                                           in_=tp)
                    if layer == 0:
                        nc.sync.dma_start(
                            out=h_chunk[b0 * P:(b0 + nb) * P, :].rearrange(
                                "(b p) f -> p b f", p=P),
                            in_=ndg[:, :nb * P].rearrange(
                                "p (b f) -> p b f", f=P),
                        )
                    else:
                        nc.sync.dma_start(
                            out=out_dr.ap()[b0 * P:(b0 + nb) * P, :].rearrange(
                                "(b p) f -> p b f", p=P),
                            in_=ndg[:, :nb * D].rearrange(
                                "p (b f) -> p b f", f=D),
                        )

                if layer == 0 and "collective" in parts:
                    if one_core:
                        nc.sync.dma_start(out=h_full[0:NP, :], in_=h_chunk)
                    else:
                        nc.gpsimd.collective_compute(
                            "AllGather",
                            mybir.AluOpType.bypass,
                            replica_groups=[list(range(NC))],
                            ins=[h_chunk.opt()],
                            outs=[h_full.opt()],
                        )

    nc.compile()
    return nc


def make_in_maps(meta, x, W_l1, b_l1, W_r1, W_l2, b_l2, W_r2):
    cfg = meta.cfg
    x = np.ascontiguousarray(np.asarray(x, dtype=np.float32))
    xp = np.zeros((cfg.N, P), BF)
    xp[:, :D] = x.astype(BF)
    ident = np.eye(D, dtype=np.float32)
    common = {
        "xp": xp,
        "wl1t": np.ascontiguousarray(np.asarray(W_l1, np.float32).T),
        "wr1t": np.ascontiguousarray(np.asarray(W_r1, np.float32).T),
        "wl2t": np.ascontiguousarray(np.asarray(W_l2, np.float32).T),
        "wr2t": np.ascontiguousarray(np.asarray(W_r2, np.float32).T),
        "b1": np.asarray(b_l1, np.float32).reshape(D, 1).copy(),
        "b2": np.asarray(b_l2, np.float32).reshape(D, 1).copy(),
        "ident": ident,
    }
    in_maps = []
    for k in range(cfg.n_cores):
        xo = x[k * cfg.n_own:(k + 1) * cfg.n_own]
        xoT = np.zeros((D, cfg.n_own_pad), np.float32)
        xoT[:, meta.perm[k]] = xo.T
        in_maps.append(dict(common, xoT=xoT, idx=meta.idx[k],
                            oh8=meta.oh8[k], invb=meta.invb[k]))
    return in_maps


_CACHE = {}
_LAST_RES = None


def kernel(x, edge_index, W_l1, b_l1, W_r1, W_l2, b_l2, W_r2):
    edge_index = np.asarray(edge_index)
    x = np.asarray(x)
    cfg = Cfg(x.shape[0])
    key = hash(edge_index.tobytes())
    if key in _CACHE:
        meta, nc = _CACHE[key]
    else:
        meta = preprocess(edge_index, cfg)
        nc = build_program(meta)
        _CACHE[key] = (meta, nc)
    in_maps = make_in_maps(meta, x, W_l1, b_l1, W_r1, W_l2, b_l2, W_r2)
    res = run_bass_kernel_spmd(nc, in_maps, core_ids=list(range(cfg.n_cores)))
    global _LAST_RES
    _LAST_RES = res
    out = np.concatenate(
        [res.results[k]["out"][meta.perm[k]] for k in range(cfg.n_cores)],
        axis=0,
    )
    return out.astype(np.float32)



# revision 41
# speedup vs baseline: 6.4808x; 1.6191x over previous
"""2-layer GraphSAGE (mean aggregation) on 8 trn2 NeuronCores via Bass/Tile.

Strategy (matches the sharding hint):
  - Nodes are row-sharded across the 8 cores (6250 rows each); edges are
    partitioned by destination core.
  - Per core, edges are grouped by 128-node destination block.  Messages
    x[src] are fetched with InstDMAGatherAnt (edge-major tiles of 128) from a
    bf16 256B-padded copy of the features, and the segment-sum is computed as
    a one-hot matmul on the tensor engine (bf16, 1 cycle/row):
        aggT[64f, 128d] += msgs[128e, 64f].T @ onehot[128e, 128d]
    where onehot[e, d] = (d == dst_local[e]) is built on the vector engine
    from a broadcast iota with one tensor_scalar(is_equal) op per tile.
    The exact f32 1/deg scaling is applied at PSUM->SBUF copy time via a
    host-built [64, n] broadcast table (elementwise mult on DVE).
  - The 64x64 weights are replicated; the dense phase runs feature-major in
    f32 on rotating [64, 512] group buffers.
  - h = tanh(layer1) chunks (bf16-padded) are AllGathered between layers.
  - dma_gather indices are int16, so each gather call reads one of two row
    regions of the source, split at a core boundary so lo/hi membership is
    identical for the x-space and the padded h-space.
"""

import numpy as np
import ml_dtypes

import concourse.bacc as bacc
import concourse.mybir as mybir
import concourse.tile as tile
from concourse.bass_utils import run_bass_kernel_spmd

P = 128
D = 64
F32 = mybir.dt.float32
BF16 = mybir.dt.bfloat16
FP8 = mybir.dt.float8e4
I16 = mybir.dt.int16
BF = ml_dtypes.bfloat16
F8 = ml_dtypes.float8_e4m3


NQ = 4  # SWDGE queues (ucode max); gather calls round-robin across them


class Cfg:
    # single_packet must stay False: True wedges the device mid-NEFF with
    # elem_size=128 bf16 gathers (worker hangs up; the axon terminal dies).
    def __init__(self, N, n_cores=8, chunk=34, msgs_bufs=6,
                 rebalance=True, sort_src=True, single_packet=False,
                 extra_blocks=1):
        assert N % n_cores == 0
        self.N = N
        self.n_cores = n_cores
        self.n_own = N // n_cores
        self.nblk = -(-self.n_own // P) + (extra_blocks if rebalance else 0)
        self.n_own_pad = self.nblk * P
        self.n_pad_all = self.n_own_pad * n_cores
        # lo/hi split at a core boundary so that edge region membership is
        # identical for x-space (N rows) and padded h-space (n_pad_all rows).
        c = n_cores // 2
        while self.N - c * self.n_own > 32768 or self.n_pad_all - c * self.n_own_pad > 32768:
            c += 1
        assert c * self.n_own <= 32768 and c * self.n_own_pad <= 32768
        self.split_core = c
        self.split = c * self.n_own
        self.split_pad = c * self.n_own_pad
        self.chunk = chunk
        self.msgs_bufs = msgs_bufs
        self.rebalance = rebalance
        self.sort_src = sort_src
        self.single_packet = single_packet


class Meta:
    pass


def _wrap16(v):
    """slot i -> [i % 16, i // 16] layout used by dma_gather idx tables."""
    assert v.shape[0] % 16 == 0
    return np.ascontiguousarray(v.reshape(-1, 16).T)


def _pack_blocks(a, b, nbins, cap_rows, n_rows_total):
    """Greedy weighted-target LPT: assign items (weights a[i], b[i]) to nbins
    bins (row capacity cap_rows, last bin n_rows_total-(nbins-1)*cap_rows).
    Bins 0..nbins-2 aim just under 8*128 edges per region so their gather
    tiles stay at 8; the last bin absorbs the remainder.  Returns bin id per
    item."""
    n = a.shape[0]
    order = np.argsort(-(a + b), kind="stable")
    caps = np.full(nbins, cap_rows, np.int64)
    target = 8 * P - 8  # 1016: leave slack below the 8-tile ceiling
    tA = np.full(nbins, target, np.float64)
    tA[-1] = max(float(a.sum() - (nbins - 1) * target), 200.0)
    tB = np.full(nbins, target, np.float64)
    tB[-1] = max(float(b.sum() - (nbins - 1) * target), 200.0)
    binA = np.zeros(nbins, np.int64)
    binB = np.zeros(nbins, np.int64)
    binc = np.zeros(nbins, np.int64)
    bin_of = np.empty(n, np.int64)
    for i in order:
        av, bv = a[i], b[i]
        load = np.maximum((binA + av) / tA, (binB + bv) / tB)
        load[binc >= caps] = np.inf
        j = int(np.argmin(load))
        bin_of[i] = j
        binA[j] += av
        binB[j] += bv
        binc[j] += 1
    return bin_of


def preprocess(edge_index, cfg):
    """Partition/group edges; build per-core gather index + onehot tables."""
    src = np.asarray(edge_index[0], dtype=np.int64)
    dst = np.asarray(edge_index[1], dtype=np.int64)
    E = src.shape[0]
    NC, NBLK = cfg.n_cores, cfg.nblk

    cnt = np.bincount(dst, minlength=cfg.N).astype(np.float32)
    inv = (1.0 / np.maximum(cnt, 1.0)).astype(np.float32)

    core = dst // cfg.n_own
    dstl = dst - core * cfg.n_own
    region = (src >= cfg.split).astype(np.int64)

    if cfg.rebalance:
        # Per-core permutation of destination rows so that per-(block,
        # region) edge counts stay near-uniform -> fewer padded gather tiles.
        deg = np.zeros((cfg.N, 2), np.int64)
        np.add.at(deg, (dst, region), 1)
        perms = []
        for k in range(NC):
            dl = deg[k * cfg.n_own:(k + 1) * cfg.n_own]
            bin_of = _pack_blocks(dl[:, 0], dl[:, 1], NBLK, P, cfg.n_own)
            perm = np.empty(cfg.n_own, np.int64)
            row = 0
            for bb in range(NBLK):
                members = np.nonzero(bin_of == bb)[0]
                perm[members] = bb * P + np.arange(len(members))
            perms.append(perm)
        meta_perm = perms
        pa = np.concatenate(perms)  # indexed by global node id
        dstl = pa[dst]
    else:
        meta_perm = [np.arange(cfg.n_own) for _ in range(NC)]

    blk = dstl // P
    inb = dstl - blk * P

    key = ((core * NBLK) + blk) * 2 + region
    ngroups = NC * NBLK * 2
    gcnt = np.bincount(key, minlength=ngroups).reshape(NC, NBLK, 2)
    # uniform (max over cores) tile counts per (block, region)
    TL = np.maximum(1, -(-gcnt[:, :, 0].max(axis=0) // P))
    TH = np.maximum(1, -(-gcnt[:, :, 1].max(axis=0) // P))
    lo_off = np.concatenate([[0], np.cumsum(TL)])
    hi_off = np.concatenate([[0], np.cumsum(TH)])
    TLT, THT = int(lo_off[-1]), int(hi_off[-1])
    T_ALL = TLT + THT

    # rank of each edge within its (core, blk, region) group; sort_src makes
    # slot order follow ascending source row for HBM locality
    if cfg.sort_src:
        order = np.lexsort((src, key))
    else:
        order = np.argsort(key, kind="stable")
    gstart = np.concatenate([[0], np.cumsum(np.bincount(key, minlength=ngroups))])[:-1]
    rank = np.empty(E, dtype=np.int64)
    rank[order] = np.arange(E) - gstart[key[order]]

    # slot within region (tiles of 128)
    reg_base = np.where(region == 0, lo_off[blk], hi_off[blk])
    slot = reg_base * P + rank

    # h-space (padded) position of each source node (via the dest perm of the
    # source's owner core)
    pa_all = np.concatenate(meta_perm)
    pos = (src // cfg.n_own) * cfg.n_own_pad + pa_all[src]

    meta = Meta()
    meta.cfg = cfg
    meta.perm = meta_perm
    meta.TL, meta.TH = TL, TH
    meta.TLT, meta.THT, meta.T_ALL = TLT, THT, T_ALL
    meta.block_tiles = [
        list(range(int(lo_off[b]), int(lo_off[b + 1])))
        + [TLT + t for t in range(int(hi_off[b]), int(hi_off[b + 1]))]
        for b in range(NBLK)
    ]

    # per-core tables
    meta.idx = []   # [128, 8*(TLT+THT)*2] int16 : l1lo | l1hi | l2lo | l2hi
    meta.dstf = []  # [128, T_ALL] f32
    meta.oh8 = []   # [128, T_ALL*128] fp8 one-hot tiles
    meta.invb = []  # [64, n_own_pad] f32 : 1/deg broadcast down 64 partitions
    for k in range(NC):
        m = core == k
        sl = slot[m]
        rg = region[m]
        s_lo, s_hi = sl[rg == 0], sl[rg == 1]
        i1lo = np.zeros(TLT * P, np.int16)
        i1hi = np.zeros(THT * P, np.int16)
        i2lo = np.zeros(TLT * P, np.int16)
        i2hi = np.zeros(THT * P, np.int16)
        i1lo[s_lo] = src[m][rg == 0]
        i1hi[s_hi] = src[m][rg == 1] - cfg.split
        i2lo[s_lo] = pos[m][rg == 0]
        i2hi[s_hi] = pos[m][rg == 1] - cfg.split_pad
        w = np.concatenate(
            [_wrap16(a) for a in (i1lo, i1hi, i2lo, i2hi)], axis=1)
        # the gather ucode reads each Q7 core's idx stripe from its own
        # 16-partition group -> replicate 8x down the partition axis
        meta.idx.append(np.ascontiguousarray(np.tile(w, (8, 1))))

        df = np.full(T_ALL * P, -1.0, np.float32)
        gs = np.where(rg == 0, 0, TLT * P) + sl
        df[gs] = inb[m].astype(np.float32)
        meta.dstf.append(np.ascontiguousarray(df.reshape(T_ALL, P).T))
        # precomputed one-hot tiles (fp8: 0/1 exact): [p, t*128+d]
        dfr = df.reshape(T_ALL, P)
        oh = dfr[:, :, None] == np.arange(P, dtype=np.float32)[None, None, :]
        meta.oh8.append(np.ascontiguousarray(
            oh.transpose(1, 0, 2).reshape(P, T_ALL * P).astype(F8)))

        iv = np.ones(cfg.n_own_pad, np.float32)
        iv[meta_perm[k]] = inv[k * cfg.n_own:(k + 1) * cfg.n_own]
        meta.invb.append(np.ascontiguousarray(np.tile(iv, (D, 1)).astype(BF)))

    meta.idx_off = [0, TLT * 8, (TLT + THT) * 8, (2 * TLT + THT) * 8]

    # gather calls: (region, t0, ntiles, first_block), interleaved by the
    # first destination block each chunk serves.  The TOTAL number of calls
    # per layer must be a multiple of 8: Tile assigns DMA-completion sems
    # round-robin over 8 DMASW lanes while queues rotate mod NQ — a call
    # count ==0 mod 8 keeps every sem pinned to one queue (same-sem calls
    # then always share a queue and complete FIFO; otherwise two queues can
    # bump one sem out of order and a consumer wait can fire early).
    def chunks(T_total, ncalls, offs):
        assert T_total >= ncalls
        base, rem = divmod(T_total, ncalls)
        out = []
        t0 = 0
        for i in range(ncalls):
            nt = base + (1 if i < rem else 0)
            fb = int(np.searchsorted(offs, t0, side="right") - 1)
            out.append((t0, nt, fb))
            t0 += nt
        assert t0 == T_total
        return out

    ncalls_lo = -(-TLT // cfg.chunk)
    ncalls_hi = -(-THT // cfg.chunk)
    pad = -(ncalls_lo + ncalls_hi) % 8
    ncalls_lo += pad // 2 + pad % 2
    ncalls_hi += pad // 2
    calls = [(0, t0, nt, fb) for (t0, nt, fb) in chunks(TLT, ncalls_lo, lo_off)]
    calls += [(1, t0, nt, fb) for (t0, nt, fb) in chunks(THT, ncalls_hi, hi_off)]
    calls.sort(key=lambda c: (c[3], c[0]))
    meta.calls = calls
    return meta


GCOL = 512  # dense-phase group width (one PSUM bank)


def build_program(meta, one_core=False,
                  parts=("gather", "agg", "dense", "store", "collective"),
                  reps=1):
    cfg = meta.cfg
    NC, NBLK = cfg.n_cores, cfg.nblk
    NP = cfg.n_own_pad
    BPG = GCOL // P  # blocks per dense group
    nc = bacc.Bacc(
        "TRN2", target_bir_lowering=False, debug=False,
        num_devices=1 if one_core else NC,
        num_swdge_queues=NQ,
    )

    xp_dr = nc.dram_tensor("xp", [cfg.N, P], BF16, kind="ExternalInput")
    xoT_dr = nc.dram_tensor("xoT", [D, NP], F32, kind="ExternalInput")
    idx_dr = nc.dram_tensor("idx", list(meta.idx[0].shape), I16, kind="ExternalInput")
    oh8_dr = nc.dram_tensor("oh8", [P, meta.T_ALL * P], FP8, kind="ExternalInput")
    invb_dr = nc.dram_tensor("invb", [D, NP], BF16, kind="ExternalInput")
    wl1_dr = nc.dram_tensor("wl1t", [D, D], F32, kind="ExternalInput")
    wr1_dr = nc.dram_tensor("wr1t", [D, D], F32, kind="ExternalInput")
    wl2_dr = nc.dram_tensor("wl2t", [D, D], F32, kind="ExternalInput")
    wr2_dr = nc.dram_tensor("wr2t", [D, D], F32, kind="ExternalInput")
    b1_dr = nc.dram_tensor("b1", [D, 1], F32, kind="ExternalInput")
    b2_dr = nc.dram_tensor("b2", [D, 1], F32, kind="ExternalInput")
    id_dr = nc.dram_tensor("ident", [D, D], F32, kind="ExternalInput")
    out_dr = nc.dram_tensor("out", [NP, D], F32, kind="ExternalOutput")

    with tile.TileContext(nc) as tc:
        with (
            tc.tile_pool(name="const", bufs=1) as cpool,
            tc.tile_pool(name="big", bufs=1) as bpool,
            tc.tile_pool(name="msgs", bufs=cfg.msgs_bufs) as mpool,
            tc.tile_pool(name="idxp", bufs=8) as ipool,
            tc.tile_pool(name="ohp", bufs=cfg.msgs_bufs) as ohpool,
            tc.tile_pool(name="grp", bufs=2) as gpool,
            tc.tile_pool(name="psA", bufs=4, space="PSUM") as psA,
            tc.tile_pool(name="psZ", bufs=2, space="PSUM") as psZ,
            tc.tile_pool(name="psT", bufs=2, space="PSUM") as psT,
            tc.tile_pool(name="dram", bufs=1, space="DRAM") as dpool,
        ):
            def load(pool, dr, shape, name, dt=F32, tag=""):
                t = pool.tile(shape, dt, name=name, tag=tag or name)
                nc.sync.dma_start(out=t, in_=dr.ap())
                return t

            ident_sb = load(cpool, id_dr, [D, D], "ident_sb")
            wl1_sb = load(cpool, wl1_dr, [D, D], "wl1_sb")
            wr1_sb = load(cpool, wr1_dr, [D, D], "wr1_sb")
            wl2_sb = load(cpool, wl2_dr, [D, D], "wl2_sb")
            wr2_sb = load(cpool, wr2_dr, [D, D], "wr2_sb")
            b1_sb = load(cpool, b1_dr, [D, 1], "b1_sb")
            b2_sb = load(cpool, b2_dr, [D, 1], "b2_sb")
            invb_sb = load(bpool, invb_dr, [D, NP], "invb_sb", dt=BF16)
            xoT_sb = load(bpool, xoT_dr, [D, NP], "xoT_sb")
            # double-buffered across reps so rep r+1's layer 1 (which writes
            # hT) can overlap rep r's layer 2 (which reads it)
            hT_bufs = [bpool.tile([D, NP], F32, name=f"hT_sb{i}", tag=f"hT{i}")
                       for i in range(min(2, reps))]

            for rep in range(reps):
              hT_sb = hT_bufs[rep % len(hT_bufs)]
              h_chunk = dpool.tile([NP, P], BF16, name=f"h_chunk_{rep}", tag=f"hc{rep}")
              h_full = dpool.tile([cfg.n_pad_all, P], BF16, name=f"h_full_{rep}",
                                  tag=f"hf{rep}", addr_space="Shared")
              for layer in range(2):
                if layer == 0:
                    src_lo = xp_dr.ap()[0:cfg.split, :]
                    src_hi = xp_dr.ap()[cfg.split:cfg.N, :]
                    off_lo, off_hi = meta.idx_off[0], meta.idx_off[1]
                else:
                    src_lo = h_full[0:cfg.split_pad, :]
                    src_hi = h_full[cfg.split_pad:cfg.n_pad_all, :]
                    off_lo, off_hi = meta.idx_off[2], meta.idx_off[3]

                # ---- gather messages (bf16, 256B rows) + stream one-hots ----
                tsrc = {}
                for ci, (rg, t0, ntile, _fb) in enumerate(meta.calls):
                    mt = mpool.tile([P, cfg.chunk, P], BF16, tag="msgs",
                                    name=f"m_{layer}_{ci}")
                    ohl = ohpool.tile([P, cfg.chunk * P], FP8, tag="oh",
                                      name=f"oh_{layer}_{ci}")
                    base = t0 if rg == 0 else meta.TLT + t0
                    if "agg" in parts:
                        nc.sync.dma_start(
                            out=ohl[:, :ntile * P],
                            in_=oh8_dr.ap()[:, base * P:(base + ntile) * P])
                    if "gather" in parts:
                        it = ipool.tile([P, cfg.chunk * 8], I16, tag="idx",
                                        name=f"i_{layer}_{ci}")
                        cols = ntile * 8
                        coff = (off_lo if rg == 0 else off_hi) + t0 * 8
                        nc.sync.dma_start(out=it[:, :cols],
                                          in_=idx_dr.ap()[:, coff:coff + cols])
                        nc.gpsimd.dma_gather(
                            mt[:, :ntile, :],
                            src_lo if rg == 0 else src_hi,
                            it[:, :cols],
                            num_idxs=ntile * P,
                            num_idxs_reg=ntile * P,
                            elem_size=P,
                            single_packet=cfg.single_packet,
                            queue_num=ci % NQ,
                        )
                    for j in range(ntile):
                        tsrc[base + j] = (mt, j, ohl)

                # ---- blocks: onehot matmul segment-sum + dense per group ----
                if layer == 0:
                    wl_sb, wr_sb, bb_sb = wl1_sb, wr1_sb, b1_sb
                    own_sb = xoT_sb
                    func = mybir.ActivationFunctionType.Tanh
                else:
                    wl_sb, wr_sb, bb_sb = wl2_sb, wr2_sb, b2_sb
                    own_sb = hT_sb
                    func = mybir.ActivationFunctionType.Identity

                ngrp = -(-NBLK // BPG)
                for g in range(ngrp if "agg" in parts else 0):
                    b0 = g * BPG
                    nb = min(BPG, NBLK - b0)
                    w = nb * P
                    aggT = gpool.tile([D, GCOL], F32, tag="aggT",
                                      name=f"agg_{rep}_{layer}_{g}")
                    for bi in range(nb):
                        b = b0 + bi
                        ps = psA.tile([D, P], F32, tag="agg", name=f"ps_{layer}_{b}")
                        gts = meta.block_tiles[b]
                        for j, gt in enumerate(gts):
                            mt, lt, ohl = tsrc[gt]
                            nc.tensor.matmul(
                                ps, lhsT=mt[:, lt, 0:D],
                                rhs=ohl[:, lt * P:(lt + 1) * P],
                                start=(j == 0), stop=(j == len(gts) - 1),
                            )
                        # exact mean scaling: psum * (1/deg) broadcast table
                        nc.vector.tensor_tensor(
                            out=aggT[:, bi * P:(bi + 1) * P], in0=ps,
                            in1=invb_sb[:, b * P:(b + 1) * P],
                            op=mybir.AluOpType.mult,
                        )
                    if "dense" not in parts:
                        continue
                    zp = psZ.tile([D, GCOL], F32, tag="z", name=f"z_{layer}_{g}")
                    nc.tensor.matmul(zp[:, :w], lhsT=wl_sb, rhs=aggT[:, :w],
                                     start=True, stop=False)
                    nc.tensor.matmul(zp[:, :w], lhsT=wr_sb,
                                     rhs=own_sb[:, b0 * P:b0 * P + w],
                                     start=False, stop=True)
                    if layer == 0:
                        outT = hT_sb
                        nc.scalar.activation(out=hT_sb[:, b0 * P:b0 * P + w],
                                             in_=zp[:, :w], func=func,
                                             bias=bb_sb[:, 0:1], scale=1.0)
                    else:
                        outT = gpool.tile([D, GCOL], F32, tag="outT",
                                          name=f"oT_{rep}_{g}")
                        nc.scalar.activation(out=outT[:, :w], in_=zp[:, :w],
                                             func=func, bias=bb_sb[:, 0:1],
                                             scale=1.0)
                    if "store" not in parts:
                        continue
                    # per-group node-major staging + store (h pad columns are
                    # never consumed downstream, so they may hold garbage)
                    if layer == 0:
                        ndg = gpool.tile([P, BPG * P], BF16, tag="ndh",
                                         name=f"ndh_{rep}_{g}")
                    else:
                        ndg = gpool.tile([P, BPG * D], F32, tag="ndo",
                                         name=f"ndo_{rep}_{g}")
                    for bi in range(nb):
                        b = b0 + bi
                        tp = psT.tile([P, D], F32, tag="tr", name=f"tp_{layer}_{b}")
                        sl = (slice(b * P, b * P + P) if layer == 0
                              else slice(bi * P, bi * P + P))
                        nc.tensor.transpose(out=tp, in_=outT[:, sl],
                                            identity=ident_sb)
                        if layer == 0:
                            nc.scalar.copy(out=ndg[:, bi * P:bi * P + D],
                                           in_=tp)
                        else:
                            nc.scalar.copy(out=ndg[:, bi * D:(bi + 1) * D],
                                           in_=tp)
                    if layer == 0:
                        nc.sync.dma_start(
                            out=h_chunk[b0 * P:(b0 + nb) * P, :].rearrange(
                                "(b p) f -> p b f", p=P),
                            in_=ndg[:, :nb * P].rearrange(
                                "p (b f) -> p b f", f=P),
                        )
                    else:
                        nc.sync.dma_start(
                            out=out_dr.ap()[b0 * P:(b0 + nb) * P, :].rearrange(
                                "(b p) f -> p b f", p=P),
                            in_=ndg[:, :nb * D].rearrange(
                                "p (b f) -> p b f", f=D),
                        )

                if layer == 0 and "collective" in parts:
                    if one_core:
                        nc.sync.dma_start(out=h_full[0:NP, :], in_=h_chunk)
                    else:
                        nc.gpsimd.collective_compute(
                            "AllGather",
                            mybir.AluOpType.bypass,
                            replica_groups=[list(range(NC))],
                            ins=[h_chunk.opt()],
                            outs=[h_full.opt()],
                        )

    nc.compile()
    return nc


def make_in_maps(meta, x, W_l1, b_l1, W_r1, W_l2, b_l2, W_r2):
    cfg = meta.cfg
    x = np.ascontiguousarray(np.asarray(x, dtype=np.float32))
    xp = np.zeros((cfg.N, P), BF)
    xp[:, :D] = x.astype(BF)
    ident = np.eye(D, dtype=np.float32)
    common = {
        "xp": xp,
        "wl1t": np.ascontiguousarray(np.asarray(W_l1, np.float32).T),
        "wr1t": np.ascontiguousarray(np.asarray(W_r1, np.float32).T),
        "wl2t": np.ascontiguousarray(np.asarray(W_l2, np.float32).T),
        "wr2t": np.ascontiguousarray(np.asarray(W_r2, np.float32).T),
        "b1": np.asarray(b_l1, np.float32).reshape(D, 1).copy(),
        "b2": np.asarray(b_l2, np.float32).reshape(D, 1).copy(),
        "ident": ident,
    }
    in_maps = []
    for k in range(cfg.n_cores):
        xo = x[k * cfg.n_own:(k + 1) * cfg.n_own]
        xoT = np.zeros((D, cfg.n_own_pad), np.float32)
        xoT[:, meta.perm[k]] = xo.T
        in_maps.append(dict(common, xoT=xoT, idx=meta.idx[k],
                            oh8=meta.oh8[k], invb=meta.invb[k]))
    return in_maps


_CACHE = {}
_LAST_RES = None


def kernel(x, edge_index, W_l1, b_l1, W_r1, W_l2, b_l2, W_r2):
    edge_index = np.asarray(edge_index)
    x = np.asarray(x)
    cfg = Cfg(x.shape[0])
    key = hash(edge_index.tobytes())
    if key in _CACHE:
        meta, nc = _CACHE[key]
    else:
        meta = preprocess(edge_index, cfg)
        nc = build_program(meta)
        _CACHE[key] = (meta, nc)
    in_maps = make_in_maps(meta, x, W_l1, b_l1, W_r1, W_l2, b_l2, W_r2)
    res = run_bass_kernel_spmd(nc, in_maps, core_ids=list(range(cfg.n_cores)))
    global _LAST_RES
    _LAST_RES = res
    out = np.concatenate(
        [res.results[k]["out"][meta.perm[k]] for k in range(cfg.n_cores)],
        axis=0,
    )
    return out.astype(np.float32)

